# revision 1
# baseline (speedup 1.0000x reference)
"""Bahdanau additive attention on 8 Trainium2 NeuronCores.

c[b] = softmax_t( tanh(s@W_a + h@U_a) @ v_a ) @ h[b]

Sharding: data-parallel over batch B=32 -> 4 batches per core; W_a, U_a,
v_a replicated. The host pre-casts h to bf16 and pre-transposes it to
[B, Dh, T] so the device streams contiguous dh-major slabs -- no SWDGE
cast DMA and no XBAR SBUF->SBUF transpose on the critical path.

Per-core pipeline, per (batch, t-chunk of 1024):
  1. DMA loads ht chunk [dh_lo, o, t] bf16 straight from HBM (chunk 0 is
     interleaved per-o with U_a across the sync+scalar queues to beat
     the ~30us DMA cold-start).
  2. PE mm1: scores_pre[a, t] += U_a[dh,a].T @ ht (8 dh-tiles in PSUM).
  3. ACT: tanh(psum + bias(W_a@s)) -> SBUF bf16 (per 128-a tile).
  4. PE e-dot with replicated v: lhsT = vrep[a_lo, 128 copies of v] so
     PSUM [128, t] holds e[t] replicated across all 128 partitions.
     Run per 512-t half so the downstream work starts sooner.
  5. ACT: exp(eps) -> pbc [128, t] bf16 (the p broadcast, for free),
     accum_out -> per-partition softmax denominator partials.
  6. DVE, 3 passes per half: scr = ht*pbc (2x bf16), pairwise fold-add
     (2x), then the 1x free-axis reduce on the halved input:
     cparts[dh_lo, o, slot] = sum_t ht * pbc.
  7. Finalize per batch on DVE: reduce chunk partials, reciprocal of the
     denominator (replicated per partition), scale, DMA out [dh_lo, o].

The softmax is unnormalized (scores bounded by ||v_a||_1 so exp() in f32
never overflows and no running max is needed).

Runtime notes: extended-ISA instructions need codegen_inst_isa_subclasses
before compile ("ISA wrong length" otherwise), and InstTensorTensorReduce
compiles but wedges the device on this runtime -- hence the 3-pass DVE.
"""

import numpy as np

B, T, DH, DS, DA = 32, 4096, 1024, 1024, 512
NCORES = 8
BL = B // NCORES          # batches per core
CHUNK_T = 1024            # timesteps per pipeline chunk
P = 128
OD = DH // P              # dh tiles (8)
AT = DA // P              # a tiles (4)

_CACHE = {}


def _legalize_waits(nc):
    """This walrus build allows at most one sync wait per instruction.
    Tile's tail drain (and any instruction whose operands arrive via two
    DMA lanes) can carry several; split the extras onto single-wait nops
    emitted just before, in the same engine's stream."""
    from concourse import mybir

    eng_map = {}
    for eng_name in ("sync", "tensor", "vector", "scalar", "gpsimd"):
        eng = getattr(nc, eng_name)
        eng_map[eng.engine] = eng

    def make_nop(engine_type):
        bi = eng_map[engine_type].nop(nofuse=True)
        inst = bi.ins
        # pop it from whatever block it was appended to
        for fn in nc.m.functions:
            for blk in fn.blocks:
                il = list(blk.instructions)
                if il and il[-1].name == inst.name:
                    blk.instructions = il[:-1]
                    return inst
        raise RuntimeError("nop not found after emit")

    for fn in nc.m.functions:
        for blk in fn.blocks:
            insts = list(blk.instructions)
            if not any(
                getattr(i, "sync_info", None) is not None
                and len(i.sync_info.on_wait) > 1
                for i in insts
            ):
                continue
            out = []
            for inst in insts:
                si = getattr(inst, "sync_info", None)
                if si is not None and len(si.on_wait) > 1:
                    waits = list(si.on_wait)
                    for w in waits[:-1]:
                        nop = make_nop(inst.engine)
                        nop.sync_info = mybir.SyncInfo(
                            on_wait=[w], on_update=[]
                        )
                        out.append(nop)
                    inst.sync_info = mybir.SyncInfo(
                        on_wait=[waits[-1]], on_update=list(si.on_update)
                    )
                out.append(inst)
            blk.instructions = out


def build_bass(bl=BL, t_total=T):
    import concourse.bass as bass
    import concourse.tile as tile
    from concourse import mybir

    f32 = mybir.dt.float32
    bf16 = mybir.dt.bfloat16
    fp8 = mybir.dt.float8e4
    Alu = mybir.AluOpType
    Act = mybir.ActivationFunctionType
    Axis = mybir.AxisListType
    DR = mybir.MatmulPerfMode.DoubleRow
    NF8 = 2                    # o-slices of the dh contraction run in fp8
    KBF = OD - NF8             # bf16 o-slices (0..KBF-1)

    nchunk = t_total // CHUNK_T

    nc = bass.Bass()
    s_ext = nc.declare_dram_parameter("s", [bl, DS], f32, isOutput=False)
    # host-side pre-transposed, pre-cast: ht[b, dh, t]
    ht_ext = nc.declare_dram_parameter(
        "h", [bl, DH, t_total], bf16, isOutput=False
    )
    w_ext = nc.declare_dram_parameter("W_a", [DS, DA], bf16, isOutput=False)
    u_ext = nc.declare_dram_parameter("U_a", [DH, DA], bf16, isOutput=False)
    v_ext = nc.declare_dram_parameter("v_a", [DA], f32, isOutput=False)
    # out[b, p, o] with dh = o*128 + p (host untangles)
    out_ext = nc.declare_dram_parameter("out", [bl, P, OD], f32, isOutput=True)

    with tile.TileContext(nc) as tc:
        from contextlib import ExitStack

        with ExitStack() as ctx:
            singles = ctx.enter_context(tc.tile_pool(name="singles", bufs=1))
            htpool = ctx.enter_context(tc.tile_pool(name="htpool", bufs=5))
            ht8pool = ctx.enter_context(tc.tile_pool(name="ht8pool", bufs=5))
            tanhpool = ctx.enter_context(tc.tile_pool(name="tanhpool", bufs=8))
            pbcpool = ctx.enter_context(tc.tile_pool(name="pbcpool", bufs=3))
            scrpool = ctx.enter_context(tc.tile_pool(name="scrpool", bufs=2))
            accpool = ctx.enter_context(tc.tile_pool(name="accpool", bufs=2))
            outpool = ctx.enter_context(tc.tile_pool(name="outpool", bufs=2))
            mm1ps = ctx.enter_context(
                tc.tile_pool(name="mm1ps", bufs=2, space="PSUM")
            )
            eps_pool = ctx.enter_context(
                tc.tile_pool(name="epsp", bufs=2, space="PSUM")
            )

            def emit_cast8(ht):
                # fp8 copy of the last NF8 o-slices for the DoubleRow tail
                # of the score matmul. Emitted with the load, chunks ahead,
                # so the DVE processes it well before the PE needs it.
                ht8 = ht8pool.tile([P, NF8, CHUNK_T], fp8, tag="ht8")
                nc.vector.tensor_copy(ht8, ht[:, KBF:OD, :])
                return ht8

            def emit_load(b, i, engines=None):
                ht = htpool.tile([P, OD, CHUNK_T], bf16, tag="ht")
                src = ht_ext[b, :, i * CHUNK_T : (i + 1) * CHUNK_T].rearrange(
                    "(o p) t -> p o t", p=P
                )
                if engines is None:
                    nc.sync.dma_start(ht, src)
                else:
                    # split across idle queues (first chunk: latency wins)
                    n = len(engines)
                    for k, eng in enumerate(engines):
                        sl = slice(k * OD // n, (k + 1) * OD // n)
                        eng.dma_start(ht[:, sl, :], src[:, sl, :])
                return ht, emit_cast8(ht)

            # Head loads: DMA runs at a fraction of steady-state bandwidth
            # for the first ~30us, so interleave per-o slices of U_a and
            # chunk 0 across the sync+scalar queues -- mm1 consumes o
            # slices in order and can start as soon as pair 0 lands.
            chunks = [(b, i) for b in range(bl) for i in range(nchunk)]
            preload = {}
            u_sb = singles.tile([P, OD, DA], bf16)
            u_re = u_ext[:].rearrange("(o p) a -> p o a", p=P)
            ht0 = htpool.tile([P, OD, CHUNK_T], bf16, tag="ht")
            ht0_src = ht_ext[0, :, 0:CHUNK_T].rearrange("(o p) t -> p o t", p=P)
            for o in range(OD):
                eng = nc.sync if o % 2 == 0 else nc.scalar
                eng.dma_start(u_sb[:, o, :], u_re[:, o, :])
                eng.dma_start(ht0[:, o, :], ht0_src[:, o, :])
            # fp8 copies of U_a's tail slices for the DoubleRow matmuls
            u8 = singles.tile([P, NF8, DA], fp8)
            nc.vector.tensor_copy(u8, u_sb[:, KBF:OD, :])
            # chunk 0 stays all-bf16: no fp8-cast dependency at the head
            preload[chunks[0]] = (ht0, None)
            for c in chunks[1:3]:
                preload[c] = emit_load(*c)

            # ---- one-time setup (gpsimd queue, off the load path) ----
            # W_a bf16 [ds_lo, ds_hi, a] (lhsT tiles for the W_a@s matmul)
            w_sb = singles.tile([P, DS // P, DA], bf16)
            nc.gpsimd.dma_start(
                w_sb, w_ext[:].rearrange("(o p) a -> p o a", p=P)
            )
            # sT [ds_lo, ds_hi, b] via strided DMAs (16 KB, one-time),
            # then a tiny DVE cast to bf16 to match w_sb for the matmul.
            st_f32 = singles.tile([P, DS // P, bl], f32)
            with nc.allow_non_contiguous_dma(
                reason="tiny one-time s transpose"
            ):
                for b in range(bl):
                    nc.gpsimd.dma_start(
                        st_f32[:, :, b],
                        s_ext[b].rearrange("(o p) -> p o", p=P),
                    )
            st_sb = singles.tile([P, DS // P, bl], bf16)
            nc.vector.tensor_copy(st_sb, st_f32)
            # v_a as [a_lo, a_hi] f32
            v_f32 = singles.tile([P, AT], f32)
            with nc.allow_non_contiguous_dma(reason="tiny one-time v load"):
                nc.gpsimd.dma_start(
                    v_f32, v_ext[:].rearrange("(g a) -> a g", g=AT)
                )

            # vrep[a_lo, at, m] = v[at*128 + a_lo] replicated over m: the
            # e-dot lhsT whose 128 identical columns replicate e across
            # every PSUM partition.
            ones128 = singles.tile([P, P], bf16)
            nc.any.memset(ones128, 1.0)
            vrep = singles.tile([P, AT, P], bf16)
            for at in range(AT):
                nc.vector.tensor_scalar_mul(
                    vrep[:, at, :], ones128, v_f32[:, at : at + 1]
                )

            # W_a_s^T setup is deferred: the ws matmuls are emitted into
            # the PE stream between chunk 0's first and second a-tile so
            # the PE can start on chunk 0 the moment ht(0) lands instead
            # of idling behind the setup DMAs.
            ws_sb = singles.tile([P, AT, bl], f32)

            def emit_ws():
                ps_ws = eps_pool.tile([P, AT, bl], f32, tag="eps")
                for at in range(AT):
                    for o in range(DS // P):
                        nc.tensor.matmul(
                            ps_ws[:, at, :],
                            w_sb[:, o, at * P : (at + 1) * P],
                            st_sb[:, o, :],
                            start=(o == 0),
                            stop=(o == DS // P - 1),
                        )
                nc.vector.tensor_copy(ws_sb, ps_ws)

            # ---- main loop ----
            def emit_chunk(b, i, ht, ht8, lparts, cparts, post_at0=None):
                # mm1: scores_pre[a, t] in PSUM, 4 a-tiles. dh-slices
                # 0..KBF-1 in bf16; the last NF8 slices as one fp8
                # DoubleRow matmul (256-deep contraction per pass).
                # Chunk 0 runs all-bf16 (ht8 None): its fp8 cast would
                # gate the PE start on the slowest head DMA slices.
                nbf = OD if ht8 is None else KBF
                tanhs = []
                for at in range(AT):
                    ps1 = mm1ps.tile([P, CHUNK_T], f32, tag="mm1")
                    for o in range(nbf):
                        lhsT = u_sb[:, o, at * P : (at + 1) * P]
                        nc.tensor.matmul(
                            ps1[:, 0:512],
                            lhsT,
                            ht[:, o, 0:512],
                            start=(o == 0),
                            stop=(o == nbf - 1 and ht8 is None),
                        )
                        nc.tensor.matmul(
                            ps1[:, 512:1024],
                            lhsT,
                            ht[:, o, 512:1024],
                            start=(o == 0),
                            stop=(o == nbf - 1 and ht8 is None),
                        )
                    if ht8 is not None:
                        for h in range(2):
                            hs = slice(h * 512, (h + 1) * 512)
                            nc.tensor.matmul(
                                ps1[:, hs],
                                u8[:, :, at * P : (at + 1) * P],
                                ht8[:, :, hs],
                                start=False,
                                stop=True,
                                perf_mode=DR,
                            )
                    if at == 0 and post_at0 is not None:
                        post_at0()
                    tanh_sb = tanhpool.tile([P, CHUNK_T], bf16, tag="tanh")
                    nc.scalar.activation(
                        tanh_sb,
                        ps1,
                        Act.Tanh,
                        bias=ws_sb[:, at, b : b + 1],
                    )
                    tanhs.append(tanh_sb)
                    if at == 1:
                        # fold pair (T0,T1): v-scaled on ACT (slack), one
                        # DVE add -> the e-dot needs 6 MMs instead of 8.
                        vt0 = scrpool.tile([P, CHUNK_T], bf16, tag="vt0")
                        nc.scalar.mul(vt0, tanhs[0], v_f32[:, 0:1])
                        vt1 = scrpool.tile([P, CHUNK_T], bf16, tag="vt1")
                        nc.scalar.mul(vt1, tanhs[1], v_f32[:, 1:2])
                        vt01 = pbcpool.tile([P, CHUNK_T], bf16, tag="vt01")
                        nc.vector.tensor_tensor(
                            out=vt01, in0=vt0, in1=vt1, op=Alu.add
                        )

                # e-dot, replicated across partitions: eps[p, t] = e[t].
                # Half-at-a-time so exp and the DVE context work can start
                # on the first 512 timesteps while the PE finishes the
                # second half (shrinks the end-of-kernel DVE tail).
                eps = eps_pool.tile([P, CHUNK_T], f32, tag="eps")
                pbc = pbcpool.tile([P, CHUNK_T], bf16, tag="pbc")
                scr = scrpool.tile([P, OD, CHUNK_T], bf16, tag="scr")
                for h in range(2):
                    hs = slice(h * 512, (h + 1) * 512)
                    edot_ops = [
                        (vrep[:, 2, :], tanhs[2]),
                        (vrep[:, 3, :], tanhs[3]),
                        (ones128, vt01),
                    ]
                    for k, (lhsT, rhs) in enumerate(edot_ops):
                        nc.tensor.matmul(
                            eps[:, hs],
                            lhsT,
                            rhs[:, hs],
                            start=(k == 0),
                            stop=(k == len(edot_ops) - 1),
                        )
                    # exp -> broadcast p [128, t] bf16 + denom partials
                    nc.scalar.activation(
                        pbc[:, hs],
                        eps[:, hs],
                        Act.Exp,
                        accum_out=lparts[:, 2 * i + h : 2 * i + h + 1],
                    )
                    # context: cparts[p, o, 2i+h] = sum_t ht[p,o,t]*p[t].
                    # DVE 3 passes: mult (2x bf16), pairwise fold-add (2x)
                    # to halve the input of the final reduce, which only
                    # runs at 1x. (InstTensorTensorReduce wedges this
                    # runtime, so no single-pass fused option.)
                    pbc3 = pbc[:, hs].rearrange(
                        "p (o t) -> p o t", o=1
                    ).broadcast_to((P, OD, 512))
                    nc.vector.tensor_tensor(
                        out=scr[:, :, hs], in0=ht[:, :, hs], in1=pbc3,
                        op=Alu.mult,
                    )
                    # two fold-add levels at 2x before the 1x reduce
                    scrf = scrpool.tile([P, OD, 256], bf16, tag="scrf")
                    lo = slice(h * 512, h * 512 + 256)
                    hi = slice(h * 512 + 256, h * 512 + 512)
                    nc.vector.tensor_tensor(
                        out=scrf, in0=scr[:, :, lo], in1=scr[:, :, hi],
                        op=Alu.add,
                    )
                    scrf2 = scrpool.tile([P, OD, 128], bf16, tag="scrf2")
                    nc.vector.tensor_tensor(
                        out=scrf2, in0=scrf[:, :, 0:128],
                        in1=scrf[:, :, 128:256], op=Alu.add,
                    )
                    nc.vector.tensor_reduce(
                        out=cparts[:, :, 2 * i + h : 2 * i + h + 1],
                        in_=scrf2,
                        axis=Axis.X,
                        op=Alu.add,
                    )

            def emit_chunk_split(b, i, ht, ht8, lparts, cparts):
                # Final chunk: process as two independent 512-t passes so
                # the second half's DVE context work is all that remains
                # after the PE finishes (halves the end-of-kernel tail).
                for h in range(2):
                    hs = slice(h * 512, (h + 1) * 512)
                    tanhs = []
                    for at in range(AT):
                        ps1 = mm1ps.tile([P, 512], f32, tag="mm1")
                        for o in range(KBF):
                            nc.tensor.matmul(
                                ps1,
                                u_sb[:, o, at * P : (at + 1) * P],
                                ht[:, o, hs],
                                start=(o == 0),
                                stop=False,
                            )
                        nc.tensor.matmul(
                            ps1,
                            u8[:, :, at * P : (at + 1) * P],
                            ht8[:, :, hs],
                            start=False,
                            stop=True,
                            perf_mode=DR,
                        )
                        tanh_sb = tanhpool.tile([P, 512], bf16, tag="tanh")
                        nc.scalar.activation(
                            tanh_sb, ps1, Act.Tanh,
                            bias=ws_sb[:, at, b : b + 1],
                        )
                        tanhs.append(tanh_sb)
                    eps = eps_pool.tile([P, 512], f32, tag="eps")
                    for at in range(AT):
                        nc.tensor.matmul(
                            eps, vrep[:, at, :], tanhs[at],
                            start=(at == 0), stop=(at == AT - 1),
                        )
                    pbc = pbcpool.tile([P, 512], bf16, tag="pbc")
                    nc.scalar.activation(
                        pbc, eps, Act.Exp,
                        accum_out=lparts[:, 2 * i + h : 2 * i + h + 1],
                    )
                    scr = scrpool.tile([P, OD, 512], bf16, tag="scr")
                    pbc3 = pbc.rearrange(
                        "p (o t) -> p o t", o=1
                    ).broadcast_to((P, OD, 512))
                    nc.vector.tensor_tensor(
                        out=scr, in0=ht[:, :, hs], in1=pbc3, op=Alu.mult
                    )
                    scrf = scrpool.tile([P, OD, 256], bf16, tag="scrf")
                    nc.vector.tensor_tensor(
                        out=scrf, in0=scr[:, :, 0:256],
                        in1=scr[:, :, 256:512], op=Alu.add,
                    )
                    scrf2 = scrpool.tile([P, OD, 128], bf16, tag="scrf2")
                    nc.vector.tensor_tensor(
                        out=scrf2, in0=scrf[:, :, 0:128],
                        in1=scrf[:, :, 128:256], op=Alu.add,
                    )
                    nc.vector.tensor_reduce(
                        out=cparts[:, :, 2 * i + h : 2 * i + h + 1],
                        in_=scrf2,
                        axis=Axis.X,
                        op=Alu.add,
                    )

            def emit_finalize(b, lparts, cparts):
                csum = outpool.tile([P, OD], f32, tag="csum")
                nc.vector.tensor_reduce(
                    out=csum, in_=cparts, axis=Axis.X, op=Alu.add
                )
                lsum = outpool.tile([P, 1], f32, tag="lsum")
                nc.vector.tensor_reduce(
                    out=lsum, in_=lparts, axis=Axis.X, op=Alu.add
                )
                rl = outpool.tile([P, 1], f32, tag="rl")
                nc.vector.reciprocal(rl, lsum)
                o_sb = outpool.tile([P, OD], f32, tag="osb")
                nc.vector.tensor_scalar_mul(o_sb, csum, rl)
                nc.scalar.dma_start(out_ext[b], o_sb)

            for idx, (b, i) in enumerate(chunks):
                if i == 0:
                    lparts = accpool.tile([P, 2 * nchunk], f32, tag="lparts")
                    cparts = accpool.tile(
                        [P, OD, 2 * nchunk], f32, tag="cparts"
                    )
                # keep the load pipeline 3 chunks ahead
                la = idx + 3
                if la < len(chunks) and chunks[la] not in preload:
                    preload[chunks[la]] = emit_load(*chunks[la])
                ht, ht8 = preload.pop((b, i))
                if idx == len(chunks) - 1 and ht8 is not None:
                    emit_chunk_split(b, i, ht, ht8, lparts, cparts)
                else:
                    emit_chunk(
                        b, i, ht, ht8, lparts, cparts,
                        post_at0=emit_ws if idx == 0 else None,
                    )
                if i == nchunk - 1:
                    emit_finalize(b, lparts, cparts)

    # Populate .instr bytes for extended-inst InstISA subclasses
    # (InstTensorTensorReduce etc.) -- raw Bass doesn't run this pass and
    # the NEFF compiler fails with "ISA wrong length" without it.
    mybir.codegen_inst_isa_subclasses(nc)
    _legalize_waits(nc)
    return nc


def _get_nc():
    if "nc" not in _CACHE:
        _CACHE["nc"] = build_bass()
    return _CACHE["nc"]


def prep_inputs(s, h, W_a, U_a, v_a):
    """Host-side prep: shard over cores, pre-cast h/U_a to bf16 and
    pre-transpose h to [b, dh, t]."""
    import ml_dtypes

    bf16 = ml_dtypes.bfloat16
    s = np.ascontiguousarray(np.asarray(s, dtype=np.float32))
    w_bf = np.ascontiguousarray(np.asarray(W_a, dtype=np.float32).astype(bf16))
    v_a = np.ascontiguousarray(np.asarray(v_a, dtype=np.float32))
    u_bf = np.ascontiguousarray(np.asarray(U_a, dtype=np.float32).astype(bf16))
    ht = np.ascontiguousarray(
        np.asarray(h, dtype=np.float32).astype(bf16).transpose(0, 2, 1)
    )
    in_maps = []
    for c in range(NCORES):
        sl = slice(c * BL, (c + 1) * BL)
        in_maps.append(
            {"s": s[sl], "h": ht[sl], "W_a": w_bf, "U_a": u_bf, "v_a": v_a}
        )
    return in_maps


def gather_out(results):
    outs = [results[c]["out"] for c in range(NCORES)]
    full = np.concatenate(outs, axis=0)  # [B, P, OD]
    return np.ascontiguousarray(
        full.transpose(0, 2, 1).reshape(B, DH)
    ).astype(np.float32)


def kernel(s, h, W_a, U_a, v_a):
    from concourse.bass_utils import run_bass_kernel_spmd

    nc = _get_nc()
    in_maps = prep_inputs(s, h, W_a, U_a, v_a)
    res = run_bass_kernel_spmd(nc, in_maps, core_ids=list(range(NCORES)))
    return gather_out(res.results)



# revision 47
# speedup vs baseline: 1.0875x; 1.0875x over previous
"""Bahdanau additive attention on 8 Trainium2 NeuronCores.

c[b] = softmax_t( tanh(s@W_a + h@U_a) @ v_a ) @ h[b]

Sharding: data-parallel over batch B=32 -> 4 batches per core; U_a, v_a
replicated. Host-side prep (part of kernel()): h cast to bf16 and
transposed to [B, Dh, T]; the last NF8=4 dh-slices ALSO pre-cast to fp8
(h8) so the device never casts; ws = s@W_a computed on host (tiny GEMV
vs a 1MB W_a load + transpose machinery on device).

Per-core pipeline, per (batch, t-chunk of 1024):
  1. DMA: ht chunk bf16 on the sync queue, h8 on the scalar queue
     (chunk 0 is interleaved per-o with U_a across sync+scalar to beat
     the DMA cold-start; chunk 0 runs all-bf16).
  2. PE mm1: scores_pre[a, t] in PSUM, per (a-tile, 512-t half):
     4 bf16 slice-passes + 2 fp8 DoubleRow passes (2 slices each, 2x
     throughput). NF8=4 fp8 slices is the error-budget limit: measured
     1.956e-2 vs the 2e-2 gate (each extra fp8 PAIR adds ~1.3e-2 in
     quadrature; NF8=6 would be ~2.3e-2).
  3. ACT: tanh(psum + ws bias) -> bf16, then vt_at = v_at * tanh_at
     (per-partition scalar mul); DVE folds vt01 = vt0+vt1, vt23 = vt2+vt3.
  4. PE e-dot DEFERRED one chunk (flushed after the next chunk's first
     a-tile so the PE never waits on the tanh->vt chain): eps[:, half]
     accumulates ones128.T@vt01 + ones128.T@vt23 -> e[t] replicated
     across all 128 PSUM partitions.
  5. ACT: exp(eps) -> pbc [128, t] bf16 + accum_out denominator partial.
  6. DVE context chain (emitted at the END of the next chunk so that
     chunk's small vt adds sit AHEAD of this bulk work in the DVE
     queue): scr = pbc*ht (2x bf16), three pairwise fold-adds (2x),
     then the 1x free-axis reduce on the 8x-shrunk input.
  7. Finalize per batch: reduce partials, reciprocal, scale, DMA out.

Tail: the second-to-last chunk is NOT deferred (PE has end-phase slack,
DVE tail shrinks), and the last chunk runs as two independent 512-t
passes with the context in 256-t quarters.

The softmax is unnormalized (scores bounded by ||v_a||_1 so exp() in f32
never overflows and no running max is needed).

Engine balance at steady state (per 1024-t chunk, measured): PE 52
matmuls ~11.9us | DVE ~11.1us | ACT ~10.0us. Wall ~240us for 33.5MB
bf16 + 8.4MB fp8 per core.

Hard-won constraints (measured on this part):
- Pool/GpSimd bulk compute is a trap: concurrent Pool ops degrade every
  DVE op ~2.3x (shared datapath) -> Pool carries only setup DMAs.
- Matmul output cannot cross a PSUM bank: 512 f32 columns max.
- fp8 DR is 1 cycle/column (2 slices per pass), same column rate as
  bf16; no 4x mode on TRN2.
- DVE tensor_tensor runs ~2x for bf16; tensor_reduce is always 1x ->
  fold before reducing. Run-to-run HW variance is +/-10-20% (throttling).

Runtime notes: extended-ISA instructions need codegen_inst_isa_subclasses
before compile ("ISA wrong length" otherwise), and InstTensorTensorReduce
compiles but wedges the device on this runtime -- hence the fold chain.
"""

import numpy as np

B, T, DH, DS, DA = 32, 4096, 1024, 1024, 512
NCORES = 8
BL = B // NCORES          # batches per core
CHUNK_T = 1024            # timesteps per pipeline chunk
P = 128
OD = DH // P              # dh tiles (8)
AT = DA // P              # a tiles (4)

_CACHE = {}


def _legalize_waits(nc):
    """This walrus build allows at most one sync wait per instruction.
    Tile's tail drain (and any instruction whose operands arrive via two
    DMA lanes) can carry several; split the extras onto single-wait nops
    emitted just before, in the same engine's stream."""
    from concourse import mybir

    eng_map = {}
    for eng_name in ("sync", "tensor", "vector", "scalar", "gpsimd"):
        eng = getattr(nc, eng_name)
        eng_map[eng.engine] = eng

    def make_nop(engine_type):
        bi = eng_map[engine_type].nop(nofuse=True)
        inst = bi.ins
        # pop it from whatever block it was appended to
        for fn in nc.m.functions:
            for blk in fn.blocks:
                il = list(blk.instructions)
                if il and il[-1].name == inst.name:
                    blk.instructions = il[:-1]
                    return inst
        raise RuntimeError("nop not found after emit")

    for fn in nc.m.functions:
        for blk in fn.blocks:
            insts = list(blk.instructions)
            if not any(
                getattr(i, "sync_info", None) is not None
                and len(i.sync_info.on_wait) > 1
                for i in insts
            ):
                continue
            out = []
            for inst in insts:
                si = getattr(inst, "sync_info", None)
                if si is not None and len(si.on_wait) > 1:
                    waits = list(si.on_wait)
                    for w in waits[:-1]:
                        nop = make_nop(inst.engine)
                        nop.sync_info = mybir.SyncInfo(
                            on_wait=[w], on_update=[]
                        )
                        out.append(nop)
                    inst.sync_info = mybir.SyncInfo(
                        on_wait=[waits[-1]], on_update=list(si.on_update)
                    )
                out.append(inst)
            blk.instructions = out


def build_bass(bl=BL, t_total=T):
    import concourse.bass as bass
    import concourse.tile as tile
    from concourse import mybir

    f32 = mybir.dt.float32
    bf16 = mybir.dt.bfloat16
    fp8 = mybir.dt.float8e4
    Alu = mybir.AluOpType
    Act = mybir.ActivationFunctionType
    Axis = mybir.AxisListType
    DR = mybir.MatmulPerfMode.DoubleRow
    NF8 = 4                    # o-slices of the dh contraction run in fp8
    KBF = OD - NF8             # bf16 o-slices (0..KBF-1)

    nchunk = t_total // CHUNK_T

    nc = bass.Bass()
    # host-side pre-transposed, pre-cast: ht[b, dh, t]
    ht_ext = nc.declare_dram_parameter(
        "h", [bl, DH, t_total], bf16, isOutput=False
    )
    # W_a @ s computed on the host (a [bl, DA] f32 GEMV-scale job): saves
    # the 1MB W_a load, the s transpose, and the ws PE matmuls, and
    # unblocks chunk 0's first tanh immediately
    ws_ext = nc.declare_dram_parameter("ws", [bl, DA], f32, isOutput=False)
    u_ext = nc.declare_dram_parameter("U_a", [DH, DA], bf16, isOutput=False)
    v_ext = nc.declare_dram_parameter("v_a", [DA], f32, isOutput=False)
    # host-precast fp8 copy of the last NF8 dh-slices of ht (same
    # f32->bf16->fp8 double rounding the device cast produced)
    h8_ext = nc.declare_dram_parameter(
        "h8", [bl, NF8 * P, t_total], fp8, isOutput=False
    )
    # out[b, p, o] with dh = o*128 + p (host untangles)
    out_ext = nc.declare_dram_parameter("out", [bl, P, OD], f32, isOutput=True)

    with tile.TileContext(nc) as tc:
        from contextlib import ExitStack

        with ExitStack() as ctx:
            singles = ctx.enter_context(tc.tile_pool(name="singles", bufs=1))
            htpool = ctx.enter_context(tc.tile_pool(name="htpool", bufs=5))
            ht8pool = ctx.enter_context(tc.tile_pool(name="ht8pool", bufs=5))
            tanhpool = ctx.enter_context(tc.tile_pool(name="tanhpool", bufs=6))
            pbcpool = ctx.enter_context(tc.tile_pool(name="pbcpool", bufs=2))
            # context chain tiles: written+read only by the DVE (in-order
            # engine), so a single buffer is race-free and saves SBUF
            scrpool = ctx.enter_context(tc.tile_pool(name="scrpool", bufs=1))
            vtpool = ctx.enter_context(tc.tile_pool(name="vtpool", bufs=2))
            accpool = ctx.enter_context(tc.tile_pool(name="accpool", bufs=2))
            outpool = ctx.enter_context(tc.tile_pool(name="outpool", bufs=2))
            mm1ps = ctx.enter_context(
                tc.tile_pool(name="mm1ps", bufs=2, space="PSUM")
            )
            eps_pool = ctx.enter_context(
                tc.tile_pool(name="epsp", bufs=2, space="PSUM")
            )

            def emit_load(b, i, h8_eng=None):
                # fp8 tail slices come pre-cast from HBM: no on-device
                # cast at all. NOTE: Pool bulk compute is a trap on this
                # part -- concurrent Pool ops degrade every DVE op ~2.3x
                # (shared datapath); and ACT/DVE casts make those engines
                # co-bottleneck with the PE. +25% DMA bytes is cheaper.
                ht = htpool.tile([P, OD, CHUNK_T], bf16, tag="ht")
                src = ht_ext[b, :, i * CHUNK_T : (i + 1) * CHUNK_T].rearrange(
                    "(o p) t -> p o t", p=P
                )
                nc.sync.dma_start(ht, src)
                ht8 = ht8pool.tile([P, NF8, CHUNK_T], fp8, tag="ht8")
                src8 = h8_ext[b, :, i * CHUNK_T : (i + 1) * CHUNK_T].rearrange(
                    "(o p) t -> p o t", p=P
                )
                (h8_eng or nc.scalar).dma_start(ht8, src8)
                return ht, ht8

            # Head loads: DMA runs at a fraction of steady-state bandwidth
            # for the first ~30us, so interleave per-o slices of U_a and
            # chunk 0 across four engine queues -- mm1 consumes o slices
            # in order and can start as soon as pair 0 lands. Pair 0 is
            # split into quarters so the PE's first matmul dep is ~96 KB
            # instead of 384 KB.
            chunks = [(b, i) for b in range(bl) for i in range(nchunk)]
            preload = {}
            u_sb = singles.tile([P, OD, DA], bf16)
            u_re = u_ext[:].rearrange("(o p) a -> p o a", p=P)
            ht0 = htpool.tile([P, OD, CHUNK_T], bf16, tag="ht")
            ht0_src = ht_ext[0, :, 0:CHUNK_T].rearrange("(o p) t -> p o t", p=P)
            # (HWDGE queues are sync+scalar only; gpsimd carries the
            # one-time setup loads.)
            nc.sync.dma_start(u_sb[:, 0, 0:P], u_re[:, 0, 0:P])
            nc.scalar.dma_start(ht0[:, 0, 0:512], ht0_src[:, 0, 0:512])
            nc.sync.dma_start(ht0[:, 0, 512:1024], ht0_src[:, 0, 512:1024])
            nc.scalar.dma_start(u_sb[:, 0, P:DA], u_re[:, 0, P:DA])
            for o in range(1, OD):
                eng = nc.sync if o % 2 == 1 else nc.scalar
                eng.dma_start(u_sb[:, o, :], u_re[:, o, :])
                eng.dma_start(ht0[:, o, :], ht0_src[:, o, :])
            # fp8 copies of U_a's tail slices for the DoubleRow matmuls,
            # per-pair so chunk 1's first DR isn't gated on the last slice
            u8 = singles.tile([P, NF8, DA], fp8)
            for k in range(NF8 // 2):
                nc.vector.tensor_copy(
                    u8[:, 2 * k : 2 * k + 2, :],
                    u_sb[:, KBF + 2 * k : KBF + 2 * k + 2, :],
                )
            # chunk 0 stays all-bf16: no fp8 dependency at the head
            preload[chunks[0]] = (ht0, None)

            # ---- one-time setup (gpsimd queue, off the load path) ----
            # host-computed ws = W_a@s as [a_lo, a_hi, b] f32 (tiny)
            ws_sb = singles.tile([P, AT, bl], f32)
            # v_a as [a_lo, a_hi] f32
            v_f32 = singles.tile([P, AT], f32)
            with nc.allow_non_contiguous_dma(reason="tiny one-time loads"):
                for b in range(bl):
                    nc.gpsimd.dma_start(
                        ws_sb[:, :, b],
                        ws_ext[b].rearrange("(g a) -> a g", g=AT),
                    )
                nc.gpsimd.dma_start(
                    v_f32, v_ext[:].rearrange("(g a) -> a g", g=AT)
                )
            # early prefetches: h8 via the gpsimd SWDGE queue (behind the
            # setup loads) so the bf16 stream on sync isn't delayed
            # during DMA cold-start
            for c in chunks[1:3]:
                preload[c] = emit_load(*c, h8_eng=nc.gpsimd)

            # vrep[a_lo, at, m] = v[at*128 + a_lo] replicated over m: the
            # e-dot lhsT whose 128 identical columns replicate e across
            # every PSUM partition.
            ones128 = singles.tile([P, P], bf16)
            nc.any.memset(ones128, 1.0)
            vrep = singles.tile([P, AT, P], bf16)
            for at in range(AT):
                nc.vector.tensor_scalar_mul(
                    vrep[:, at, :], ones128, v_f32[:, at : at + 1]
                )

            # ---- main loop ----
            def emit_chunk(b, i, ht, ht8, lparts, cparts, post_at0=None):
                # mm1: scores_pre[a, t] in PSUM, 4 a-tiles. dh-slices
                # 0..KBF-1 in bf16; the last NF8 slices as one fp8
                # DoubleRow matmul (256-deep contraction per pass).
                # Chunk 0 runs all-bf16 (ht8 None): its fp8 cast would
                # gate the PE start on the slowest head DMA slices.
                nbf = OD if ht8 is None else KBF
                tanhs = []
                vts = []
                for at in range(AT):
                    ps1 = mm1ps.tile([P, CHUNK_T], f32, tag="mm1")
                    for o in range(nbf):
                        lhsT = u_sb[:, o, at * P : (at + 1) * P]
                        nc.tensor.matmul(
                            ps1[:, 0:512],
                            lhsT,
                            ht[:, o, 0:512],
                            start=(o == 0),
                            stop=(o == nbf - 1 and ht8 is None),
                        )
                        nc.tensor.matmul(
                            ps1[:, 512:1024],
                            lhsT,
                            ht[:, o, 512:1024],
                            start=(o == 0),
                            stop=(o == nbf - 1 and ht8 is None),
                        )
                    if ht8 is not None:
                        npair = NF8 // 2
                        for h in range(2):
                            hs = slice(h * 512, (h + 1) * 512)
                            for k in range(npair):
                                ksl = slice(2 * k, 2 * k + 2)
                                nc.tensor.matmul(
                                    ps1[:, hs],
                                    u8[:, ksl, at * P : (at + 1) * P],
                                    ht8[:, ksl, hs],
                                    start=False,
                                    stop=(k == npair - 1),
                                    perf_mode=DR,
                                )
                    tanh_sb = tanhpool.tile([P, CHUNK_T], bf16, tag="tanh")
                    nc.scalar.activation(
                        tanh_sb,
                        ps1,
                        Act.Tanh,
                        bias=ws_sb[:, at, b : b + 1],
                    )
                    tanhs.append(tanh_sb)
                    # v-squash on ACT (slack engine): vt_at = v_at*tanh_at,
                    # pair-folded on DVE -> the e-dot is 2 MMs per half.
                    vt = vtpool.tile([P, CHUNK_T], bf16, tag=f"vt{at}")
                    nc.scalar.mul(vt, tanh_sb, v_f32[:, at : at + 1])
                    vts.append(vt)
                    if at == 1:
                        vt01 = pbcpool.tile([P, CHUNK_T], bf16, tag="vt01")
                        nc.vector.tensor_tensor(
                            out=vt01, in0=vts[0], in1=vts[1], op=Alu.add
                        )
                    if at == 3:
                        vt23 = pbcpool.tile([P, CHUNK_T], bf16, tag="vt23")
                        nc.vector.tensor_tensor(
                            out=vt23, in0=vts[2], in1=vts[3], op=Alu.add
                        )
                    if at == 0 and post_at0 is not None:
                        post_at0()

                def tail_pe():
                    # e-dot, replicated across partitions: eps[p,t] = e[t].
                    # Emitted into the PE stream one chunk later (after
                    # the next chunk's first a-tile) so the PE never waits
                    # on the tanh->vt chain.
                    eps = eps_pool.tile([P, CHUNK_T], f32, tag="eps")
                    for h in range(2):
                        hs = slice(h * 512, (h + 1) * 512)
                        nc.tensor.matmul(
                            eps[:, hs], ones128, vt01[:, hs],
                            start=True, stop=False,
                        )
                        nc.tensor.matmul(
                            eps[:, hs], ones128, vt23[:, hs],
                            start=False, stop=True,
                        )
                    # exp -> broadcast p [128, t] bf16 + denom partial
                    pbc = pbcpool.tile([P, CHUNK_T], bf16, tag="pbc")
                    nc.scalar.activation(
                        pbc, eps, Act.Exp, accum_out=lparts[:, i : i + 1]
                    )
                    return pbc

                def tail_dve(pbc):
                    # context: cparts[p, o, i] = sum_t ht[p,o,t]*p[t].
                    # DVE: mult (2x bf16), three fold-add levels, then the
                    # 1x free-axis reduce on the 8x-shrunk input.
                    # (InstTensorTensorReduce wedges this runtime, so no
                    # single-pass fused option.) Emitted at the END of the
                    # next chunk's emission so the next chunk's vt adds
                    # sit AHEAD of this bulk work in the DVE queue -- the
                    # PE's deferred e-dot then never waits on a backlog.
                    pbc3 = pbc.rearrange(
                        "p (o t) -> p o t", o=1
                    ).broadcast_to((P, OD, CHUNK_T))
                    scr = scrpool.tile([P, OD, CHUNK_T], bf16, tag="scr")
                    nc.vector.tensor_tensor(
                        out=scr, in0=pbc3, in1=ht, op=Alu.mult,
                    )
                    scrf = scrpool.tile([P, OD, 512], bf16, tag="scrf")
                    nc.vector.tensor_tensor(
                        out=scrf, in0=scr[:, :, 0:512],
                        in1=scr[:, :, 512:1024], op=Alu.add,
                    )
                    scrf2 = scrpool.tile([P, OD, 256], bf16, tag="scrf2")
                    nc.vector.tensor_tensor(
                        out=scrf2, in0=scrf[:, :, 0:256],
                        in1=scrf[:, :, 256:512], op=Alu.add,
                    )
                    scrf3 = scrpool.tile([P, OD, 128], bf16, tag="scrf3")
                    nc.vector.tensor_tensor(
                        out=scrf3, in0=scrf2[:, :, 0:128],
                        in1=scrf2[:, :, 128:256], op=Alu.add,
                    )
                    nc.vector.tensor_reduce(
                        out=cparts[:, :, i : i + 1],
                        in_=scrf3,
                        axis=Axis.X,
                        op=Alu.add,
                    )

                return tail_pe, tail_dve

            def emit_chunk_split(b, i, ht, ht8, lparts, cparts, post_first):
                # Final chunk: two independent 512-t passes, with the DVE
                # context chain further split into 256-t quarters, so the
                # post-PE drain is just one quarter-chain (~2.5us) instead
                # of a full chunk's context work (~11us).
                for h in range(2):
                    hs = slice(h * 512, (h + 1) * 512)
                    tanhs = []
                    for at in range(AT):
                        ps1 = mm1ps.tile([P, 512], f32, tag="mm1")
                        for o in range(KBF):
                            nc.tensor.matmul(
                                ps1,
                                u_sb[:, o, at * P : (at + 1) * P],
                                ht[:, o, hs],
                                start=(o == 0),
                                stop=False,
                            )
                        for k in range(NF8 // 2):
                            ksl = slice(2 * k, 2 * k + 2)
                            nc.tensor.matmul(
                                ps1,
                                u8[:, ksl, at * P : (at + 1) * P],
                                ht8[:, ksl, hs],
                                start=False,
                                stop=(k == NF8 // 2 - 1),
                                perf_mode=DR,
                            )
                        if h == 0 and at == 0 and post_first is not None:
                            post_first()
                        tanh_sb = tanhpool.tile([P, 512], bf16, tag="tanh")
                        nc.scalar.activation(
                            tanh_sb, ps1, Act.Tanh,
                            bias=ws_sb[:, at, b : b + 1],
                        )
                        tanhs.append(tanh_sb)
                    eps = eps_pool.tile([P, 512], f32, tag="eps")
                    for at in range(AT):
                        nc.tensor.matmul(
                            eps, vrep[:, at, :], tanhs[at],
                            start=(at == 0), stop=(at == AT - 1),
                        )
                    pbc = pbcpool.tile([P, 512], bf16, tag="pbc")
                    nc.scalar.activation(
                        pbc, eps, Act.Exp,
                        accum_out=lparts[:, i + h : i + h + 1],
                    )
                    for q in range(2):
                        qs = slice(h * 512 + q * 256, h * 512 + (q + 1) * 256)
                        qp = slice(q * 256, (q + 1) * 256)
                        scr = scrpool.tile([P, OD, 256], bf16, tag="scr")
                        pbc3 = pbc[:, qp].rearrange(
                            "p (o t) -> p o t", o=1
                        ).broadcast_to((P, OD, 256))
                        nc.vector.tensor_tensor(
                            out=scr, in0=ht[:, :, qs], in1=pbc3, op=Alu.mult,
                        )
                        scrf = scrpool.tile([P, OD, 128], bf16, tag="scrf")
                        nc.vector.tensor_tensor(
                            out=scrf, in0=scr[:, :, 0:128],
                            in1=scr[:, :, 128:256], op=Alu.add,
                        )
                        nc.vector.tensor_reduce(
                            out=cparts[
                                :, :, i + 2 * h + q : i + 2 * h + q + 1
                            ],
                            in_=scrf,
                            axis=Axis.X,
                            op=Alu.add,
                        )

            def emit_finalize(b, lparts, cparts, nl, ncp):
                csum = outpool.tile([P, OD], f32, tag="csum")
                nc.vector.tensor_reduce(
                    out=csum, in_=cparts[:, :, 0:ncp], axis=Axis.X, op=Alu.add
                )
                lsum = outpool.tile([P, 1], f32, tag="lsum")
                nc.vector.tensor_reduce(
                    out=lsum, in_=lparts[:, 0:nl], axis=Axis.X, op=Alu.add
                )
                rl = outpool.tile([P, 1], f32, tag="rl")
                nc.vector.reciprocal(rl, lsum)
                o_sb = outpool.tile([P, OD], f32, tag="osb")
                nc.vector.tensor_scalar_mul(o_sb, csum, rl)
                nc.scalar.dma_start(out_ext[b], o_sb)

            # pending = deferred tail of the previous chunk. The PE/ACT
            # part (e-dot + exp) flushes after the next chunk's first
            # a-tile; the DVE context part flushes at the END of the next
            # chunk's emission so that chunk's vt adds sit ahead of the
            # bulk context work in the DVE queue.
            pending = None
            deferred_dve = None

            def flush_pending():
                nonlocal pending, deferred_dve
                if pending is None:
                    return
                pb, pi, tpe, tdve, pl, pc = pending
                pending = None
                pbc = tpe()

                def run_dve():
                    tdve(pbc)
                    if pi == nchunk - 1:
                        emit_finalize(pb, pl, pc, nchunk, nchunk)

                deferred_dve = run_dve

            def flush_deferred_dve():
                nonlocal deferred_dve
                if deferred_dve is not None:
                    fn = deferred_dve
                    deferred_dve = None
                    fn()

            for idx, (b, i) in enumerate(chunks):
                if i == 0:
                    lparts = accpool.tile([P, nchunk + 1], f32, tag="lparts")
                    cparts = accpool.tile(
                        [P, OD, nchunk + 3], f32, tag="cparts"
                    )
                # keep the load pipeline 3 chunks ahead
                la = idx + 3
                if la < len(chunks) and chunks[la] not in preload:
                    preload[chunks[la]] = emit_load(*chunks[la])
                ht, ht8 = preload.pop((b, i))
                if idx == len(chunks) - 1 and ht8 is not None:
                    # flush the previous chunk's whole tail first: its
                    # e-dot/exp fire at the split chunk's start and its
                    # context drains on the DVE while the split chunk's
                    # PE work runs
                    flush_pending()
                    flush_deferred_dve()
                    emit_chunk_split(
                        b, i, ht, ht8, lparts, cparts, None
                    )
                    emit_finalize(b, lparts, cparts, nchunk + 1, nchunk + 3)
                else:
                    tail_pe, tail_dve = emit_chunk(
                        b, i, ht, ht8, lparts, cparts,
                        post_at0=flush_pending,
                    )
                    flush_deferred_dve()
                    if idx == len(chunks) - 2:
                        # second-to-last chunk: don't defer -- the PE has
                        # end-phase slack and the DVE tail shrinks ~10us
                        # when this context starts a chunk earlier
                        tail_dve(tail_pe())
                    else:
                        pending = (b, i, tail_pe, tail_dve, lparts, cparts)

    # Populate .instr bytes for extended-inst InstISA subclasses
    # (InstTensorTensorReduce etc.) -- raw Bass doesn't run this pass and
    # the NEFF compiler fails with "ISA wrong length" without it.
    mybir.codegen_inst_isa_subclasses(nc)
    _legalize_waits(nc)
    return nc


def _get_nc():
    if "nc" not in _CACHE:
        _CACHE["nc"] = build_bass()
    return _CACHE["nc"]


NF8 = 4  # must match build_bass


def prep_inputs(s, h, W_a, U_a, v_a):
    """Host-side prep: shard over cores, pre-cast h/U_a to bf16,
    pre-transpose h to [b, dh, t], and pre-cast the last NF8 dh-slices
    to fp8 (same bf16->fp8 rounding the device cast used)."""
    import ml_dtypes

    bf16 = ml_dtypes.bfloat16
    fp8 = ml_dtypes.float8_e4m3
    s = np.asarray(s, dtype=np.float32)
    W_a = np.asarray(W_a, dtype=np.float32)
    ws = np.ascontiguousarray(s @ W_a)  # [B, DA] f32, host-side
    v_a = np.ascontiguousarray(np.asarray(v_a, dtype=np.float32))
    u_bf = np.ascontiguousarray(np.asarray(U_a, dtype=np.float32).astype(bf16))
    ht = np.ascontiguousarray(
        np.asarray(h, dtype=np.float32).astype(bf16).transpose(0, 2, 1)
    )
    h8 = np.ascontiguousarray(ht[:, (OD - NF8) * 128 :, :].astype(fp8))
    in_maps = []
    for c in range(NCORES):
        sl = slice(c * BL, (c + 1) * BL)
        in_maps.append(
            {
                "ws": ws[sl], "h": ht[sl], "h8": h8[sl],
                "U_a": u_bf, "v_a": v_a,
            }
        )
    return in_maps


def gather_out(results):
    outs = [results[c]["out"] for c in range(NCORES)]
    full = np.concatenate(outs, axis=0)  # [B, P, OD]
    return np.ascontiguousarray(
        full.transpose(0, 2, 1).reshape(B, DH)
    ).astype(np.float32)


def kernel(s, h, W_a, U_a, v_a):
    from concourse.bass_utils import run_bass_kernel_spmd

    nc = _get_nc()
    in_maps = prep_inputs(s, h, W_a, U_a, v_a)
    res = run_bass_kernel_spmd(nc, in_maps, core_ids=list(range(NCORES)))
    return gather_out(res.results)

